# revision 26
# baseline (speedup 1.0000x reference)
"""GraphSAGE (3x SAGEConv mean-aggr + BN + MLP head) on 8 Trainium2 NeuronCores.

Strategy: node sharding (12500 nodes/core). Edges partitioned by dst core and
grouped by (dst-tile of 128, src-bank of 25000). Activation table is
AllGathered (fp16) each layer; per-edge source features are fetched with
gpsimd.dma_gather (4 SWDGE queues), segment-summed into PSUM via one-hot
matmuls, mean-scaled, then W_l/W_r matmuls with a folded BN+ReLU epilogue.
fp16 data path, f32 accumulation.
"""

import os
import numpy as np

from concourse import bass, bacc, mybir, tile
from concourse.bass_utils import run_bass_kernel_spmd
from concourse.library_config import mlp as MLP_LIB

# problem constants (hardcoded per spec)
N = 100000
E = 1600000
F_IN = 128
H = 256
C = 40
BN_EPS = 1e-5
NCORES = 8
SHARD = N // NCORES          # 12500
NTILE = SHARD // 128         # 98 dst tiles (the last one is 84 wide: 12500=97*128+84)
BANK = 25000                 # int16-addressable src bank size
NBANK = (N + BANK - 1) // BANK

F16 = mybir.dt.float16
F32 = mybir.dt.float32
I16 = mybir.dt.int16

LAST_EXEC_NS = None
LAST_RES = None


def _preprocess(x, edge_index, pre_w, pre_b, bn_params, lin_l_w, lin_l_b, lin_r_w,
                post1_w, post1_b, post2_w, post2_b):
    """Host-side sharding: per-core edge gather/one-hot metadata + weights."""
    ntile = (SHARD + 127) // 128
    src = np.asarray(edge_index[0], dtype=np.int64)
    dst = np.asarray(edge_index[1], dtype=np.int64)

    deg = np.bincount(dst, minlength=N).astype(np.float64)
    invdeg_full = (1.0 / np.maximum(deg, 1.0)).astype(np.float32)

    core_of = dst // SHARD
    counts = np.zeros((NCORES, ntile, NBANK), dtype=np.int64)
    per_core = []
    for c in range(NCORES):
        m = core_of == c
        s_c = src[m]
        d_c = dst[m] - c * SHARD
        t_c = d_c // 128
        b_c = s_c // BANK
        order = np.lexsort((b_c, t_c))
        s_c, d_c, t_c, b_c = s_c[order], d_c[order], t_c[order], b_c[order]
        key = t_c * NBANK + b_c
        cnt = np.bincount(key, minlength=ntile * NBANK).reshape(ntile, NBANK)
        counts[c] = cnt
        per_core.append((s_c, d_c, key))

    nchunks = (np.ceil(counts.max(axis=0) / 128.0)).astype(np.int64)  # [ntile, NBANK]
    total_chunks = int(nchunks.sum())

    idx_arr = np.zeros((NCORES, 128, total_chunks * 8), dtype=np.int16)
    dstl_arr = np.full((NCORES, 128, total_chunks), 255.0, dtype=np.float16)
    G = 1  # tiles per supertile group (shared gather calls per bank)
    chunk_base = np.zeros((ntile, NBANK), dtype=np.int64)
    acc = 0
    for g0 in range(0, ntile, G):
        for b in range(NBANK):
            for t in range(g0, min(g0 + G, ntile)):
                chunk_base[t, b] = acc
                acc += nchunks[t, b]

    for c in range(NCORES):
        s_c, d_c, _key = per_core[c]
        grp_cnt = counts[c].reshape(-1)
        grp_start = np.zeros(ntile * NBANK + 1, dtype=np.int64)
        np.cumsum(grp_cnt, out=grp_start[1:])
        for t in range(ntile):
            for b in range(NBANK):
                k = int(counts[c, t, b])
                if k == 0:
                    continue
                g0 = grp_start[t * NBANK + b]
                svals = (s_c[g0 : g0 + k] - b * BANK).astype(np.int16)
                dvals = (d_c[g0 : g0 + k] - t * 128).astype(np.float16)
                cb = int(chunk_base[t, b])
                i = np.arange(k)
                for g in range(8):
                    idx_arr[c, 16 * g + (i % 16), cb * 8 + (i // 16)] = svals
                dstl_arr[c, i % 128, cb + i // 128] = dvals
    # pads: idx stays 0 (valid row 0 of the bank), dstl stays 255 (no one-hot col)

    invd_arr = np.zeros((NCORES, 128, ntile), dtype=np.float32)
    for c in range(NCORES):
        v = np.zeros(ntile * 128, dtype=np.float32)
        v[:SHARD] = invdeg_full[c * SHARD : (c + 1) * SHARD]
        invd_arr[c, :, :] = v.reshape(ntile, 128).T

    # ---- weights (replicated) ----
    f = np.float32
    pre_w = np.asarray(pre_w, f); pre_b = np.asarray(pre_b, f)
    bn = np.asarray(bn_params, f)
    lin_l_w = np.asarray(lin_l_w, f); lin_l_b = np.asarray(lin_l_b, f)
    lin_r_w = np.asarray(lin_r_w, f)
    post1_w = np.asarray(post1_w, f); post1_b = np.asarray(post1_b, f)
    post2_w = np.asarray(post2_w, f); post2_b = np.asarray(post2_b, f)

    def lhsT2(w):  # w [fo, fi] -> [128, nfi, fo] fp16 (K-chunked transposed)
        fo, fi = w.shape
        nfi = fi // 128
        out = np.zeros((128, nfi, fo), dtype=np.float16)
        for kc in range(nfi):
            out[:, kc, :] = w[:, kc * 128 : (kc + 1) * 128].T.astype(np.float16)
        return out

    # BN folds: y -> relu(scale*y + shift)
    scales = np.zeros((5, H), f); shifts = np.zeros((5, H), f)
    biases = [pre_b, lin_l_b[0], lin_l_b[1], lin_l_b[2], post1_b]
    for k in range(5):
        g, be, m, v = bn[k, 0], bn[k, 1], bn[k, 2], bn[k, 3]
        s = g / np.sqrt(v + BN_EPS)
        scales[k] = s
        shifts[k] = s * (biases[k] - m) + be

    iota_row = np.tile(np.arange(128, dtype=np.float16)[None, :], (128, 1))
    ident16 = np.eye(128, dtype=np.float16)
    ident32 = np.eye(128, dtype=f)

    x_f16 = np.asarray(x, f).astype(np.float16)
    # pad shard to ntile*128 rows
    xpad = np.zeros((NCORES, ntile * 128, F_IN), dtype=np.float16)
    for c in range(NCORES):
        xpad[c, :SHARD] = x_f16[c * SHARD : (c + 1) * SHARD]

    common = {
        "iota_row": iota_row,
        "ident16": ident16,
        "ident32": ident32,
        "pre_wT": lhsT2(pre_w),
        "wlT": np.stack([lhsT2(lin_l_w[i]) for i in range(3)]),
        "wrT": np.stack([lhsT2(lin_r_w[i]) for i in range(3)]),
        "p1T": lhsT2(post1_w),
        "p2T": lhsT2(post2_w),
        "bn_scale": np.ascontiguousarray(scales.reshape(5, 2, 128).transpose(2, 0, 1)),
        "bn_shift": np.ascontiguousarray(shifts.reshape(5, 2, 128).transpose(2, 0, 1)),
        "p2b": np.ascontiguousarray(post2_b.reshape(C, 1)),
    }
    in_maps = []
    for c in range(NCORES):
        m = dict(common)
        m["x"] = np.ascontiguousarray(xpad[c])
        m["gidx"] = np.ascontiguousarray(idx_arr[c])
        m["dstl"] = np.ascontiguousarray(dstl_arr[c])
        m["invd"] = np.ascontiguousarray(invd_arr[c])
        in_maps.append(m)
    return nchunks, chunk_base, total_chunks, in_maps


def _build(nchunks, chunk_base, total_chunks):
    ntile = nchunks.shape[0]
    nc = bacc.Bacc("TRN2", target_bir_lowering=False, debug=False,
                   num_devices=NCORES, num_swdge_queues=4)
    dram_in = lambda n, s, d: nc.dram_tensor(n, s, d, kind="ExternalInput").ap()

    x_in = dram_in("x", [ntile * 128, F_IN], F16)
    gidx = dram_in("gidx", [128, total_chunks * 8], I16)
    dstl = dram_in("dstl", [128, total_chunks], F16)
    invd = dram_in("invd", [128, ntile], F32)
    iota_row = dram_in("iota_row", [128, 128], F16)
    ident16 = dram_in("ident16", [128, 128], F16)
    ident32 = dram_in("ident32", [128, 128], F32)
    pre_wT = dram_in("pre_wT", [128, 1, H], F16)
    wlT = dram_in("wlT", [3, 128, 2, H], F16)
    wrT = dram_in("wrT", [3, 128, 2, H], F16)
    p1T = dram_in("p1T", [128, 2, H], F16)
    p2T = dram_in("p2T", [128, 2, C], F16)
    bn_scale = dram_in("bn_scale", [128, 5, 2], F32)
    bn_shift = dram_in("bn_shift", [128, 5, 2], F32)
    p2b = dram_in("p2b", [C, 1], F32)
    out = nc.dram_tensor("out", [SHARD, C], F32, kind="ExternalOutput").ap()
    debug = bool(os.environ.get("GSAGE_DEBUG"))
    if debug:
        dbg = [nc.dram_tensor(f"dbg{i}", [SHARD, H], F16, kind="ExternalOutput").ap()
               for i in range(4)]
        dbgz = nc.dram_tensor("dbgz", [N, H], F16, kind="ExternalOutput").ap()

    Relu = mybir.ActivationFunctionType.Relu
    Copy = mybir.ActivationFunctionType.Copy
    Exp = mybir.ActivationFunctionType.Exp
    Ln = mybir.ActivationFunctionType.Ln
    Ident = mybir.ActivationFunctionType.Identity

    with tile.TileContext(nc) as tc:
        with (
            tc.tile_pool(name="const", bufs=1) as constp,
            tc.tile_pool(name="hs", bufs=1) as hsp,
            tc.tile_pool(name="gat", bufs=12) as gatp,
            tc.tile_pool(name="oh", bufs=16) as ohp,
            tc.tile_pool(name="sb", bufs=3) as sbp,
            tc.tile_pool(name="vec", bufs=4) as vecp,
            tc.tile_pool(name="psA", bufs=3, space="PSUM") as psA,
            tc.tile_pool(name="psB", bufs=2, space="PSUM") as psB,
            tc.tile_pool(name="psT", bufs=3, space="PSUM") as psT,
            tc.tile_pool(name="dram", bufs=1, space="DRAM") as dramp,
        ):
            nc.gpsimd.load_library(MLP_LIB)

            # ---- resident constants / inputs ----
            idx_sb = constp.tile([128, total_chunks * 8], I16)
            nc.sync.dma_start(out=idx_sb[:], in_=gidx[:])
            dstl_sb = constp.tile([128, total_chunks], F16)
            nc.sync.dma_start(out=dstl_sb[:], in_=dstl[:])
            invd_sb = constp.tile([128, ntile], F32)
            nc.sync.dma_start(out=invd_sb[:], in_=invd[:])
            iota_sb = constp.tile([128, 128], F16)
            nc.sync.dma_start(out=iota_sb[:], in_=iota_row[:])
            id16_sb = constp.tile([128, 128], F16)
            nc.sync.dma_start(out=id16_sb[:], in_=ident16[:])
            id32_sb = constp.tile([128, 128], F32)
            nc.sync.dma_start(out=id32_sb[:], in_=ident32[:])
            pre_wT_sb = constp.tile([128, 1, H], F16)
            nc.sync.dma_start(out=pre_wT_sb[:], in_=pre_wT[:])
            wlT_sb = [constp.tile([128, 2, H], F16, tag=f"wlT{i}", name=f"wlT{i}") for i in range(3)]
            wrT_sb = [constp.tile([128, 2, H], F16, tag=f"wrT{i}", name=f"wrT{i}") for i in range(3)]
            for i in range(3):
                nc.sync.dma_start(out=wlT_sb[i][:], in_=wlT[i])
                nc.sync.dma_start(out=wrT_sb[i][:], in_=wrT[i])
            p1T_sb = constp.tile([128, 2, H], F16)
            nc.sync.dma_start(out=p1T_sb[:], in_=p1T[:])
            p2T_sb = constp.tile([128, 2, C], F16)
            nc.sync.dma_start(out=p2T_sb[:], in_=p2T[:])
            bns_sb = constp.tile([128, 5, 2], F32)
            nc.sync.dma_start(out=bns_sb[:], in_=bn_scale[:])
            bnb_sb = constp.tile([128, 5, 2], F32)
            nc.sync.dma_start(out=bnb_sb[:], in_=bn_shift[:])
            p2b_sb = constp.tile([C, 1], F32)
            nc.sync.dma_start(out=p2b_sb[:], in_=p2b[:])

            # local shard activations, node-major fp16 [p, tile, feat]
            hs_sb = hsp.tile([128, ntile, H], F16)

            hs_dram = dramp.tile([SHARD, H], F16)
            zfull = [
                dramp.tile([N, H], F16, addr_space="Shared", tag=f"zfull{i}", name=f"zfull{i}")
                for i in range(3)
            ]


            def do_tile_out(t, terms, bnk, store_dram):
                """terms: list of (lhsT_sb [128,nfi,H], rhsT_sb [128,nfi,128]).
                Transposed matmul + fused BN+ReLU, transposed back into
                hs_sb[:, t, :]; optionally stages the row block to hs_dram."""
                po = psB.tile([128, 2, 128], F32, tag="po")
                total = 2 * sum(lh.shape[1] for lh, _ in terms)
                done = 0
                for fh in range(2):
                    for lh, rh in terms:
                        nfi = lh.shape[1]
                        for fi in range(nfi):
                            done += 1
                            nc.tensor.matmul(
                                out=po[:, fh, :],
                                lhsT=lh[:, fi, fh * 128 : (fh + 1) * 128],
                                rhs=rh[:, fi, :],
                                start=(done == 1 or (fh == 1 and done == total // 2 + 1)),
                                stop=(done == total // 2 or done == total),
                            )
                hT = sbp.tile([128, 2, 128], F16, tag="hT")
                for fh in range(2):
                    nc.scalar.activation(
                        out=hT[:, fh, :], in_=po[:, fh, :], func=Relu,
                        bias=bnb_sb[:, bnk, fh : fh + 1],
                        scale=bns_sb[:, bnk, fh : fh + 1],
                    )
                pt = psT.tile([128, 2, 128], F16, tag="pt")
                for fh in range(2):
                    nc.tensor.transpose(out=pt[:, fh, :], in_=hT[:, fh, :], identity=id16_sb[:])
                for fh in range(2):
                    nc.vector.tensor_copy(
                        out=hs_sb[:, t, fh * 128 : (fh + 1) * 128], in_=pt[:, fh, :]
                    )
                if store_dram:
                    lo = t * 128
                    n_t = min(128, SHARD - lo)
                    nc.sync.dma_start(
                        out=hs_dram[lo : lo + n_t, :], in_=hs_sb[:n_t, t, :]
                    )

            # ---------- pre layer: h = relu(bn(x @ pre_w.T + pre_b)) ----------
            for t in range(ntile):
                xt = sbp.tile([128, F_IN], F16, tag="xt")
                nc.sync.dma_start(out=xt[:], in_=x_in[t * 128 : (t + 1) * 128, :])
                ptx = psT.tile([128, 2, 128], F16, tag="pt")
                nc.tensor.transpose(out=ptx[:, 0, :], in_=xt[:], identity=id16_sb[:])
                xT = sbp.tile([128, 1, 128], F16, tag="xT")
                nc.vector.tensor_copy(out=xT[:, 0, :], in_=ptx[:, 0, :])
                do_tile_out(t, [(pre_wT_sb, xT)], 0, True)

            if debug:
                nc.sync.dma_start(out=dbg[0][:], in_=hs_dram[:])
            nc.gpsimd.collective_compute(
                "AllGather", mybir.AluOpType.bypass,
                replica_groups=[list(range(NCORES))],
                ins=[hs_dram[:].opt()], outs=[zfull[0][:].opt()],
            )


            if debug:
                for blk in range(0, N, 12500):
                    nc.sync.dma_start(out=dbgz[blk : blk + 12500, :],
                                      in_=zfull[0][blk : blk + 12500, :])
            def _conv_tile_tail(li, t, pa_t):
                mean_sb = sbp.tile([128, H], F16, tag="mean", name=f"mean{li}_{t}")
                nc.scalar.activation(
                    out=mean_sb[:], in_=pa_t, func=Copy,
                    scale=invd_sb[:, t : t + 1],
                )
                ptm = psT.tile([128, 2, 128], F16, tag="pt", name=f"ptm{li}_{t}")
                for fh in range(2):
                    nc.tensor.transpose(
                        out=ptm[:, fh, :],
                        in_=mean_sb[:, fh * 128 : (fh + 1) * 128],
                        identity=id16_sb[:],
                    )
                mT = sbp.tile([128, 2, 128], F16, tag="mT", name=f"mT{li}_{t}")
                nc.vector.tensor_copy(out=mT[:], in_=ptm[:])
                ptz = psT.tile([128, 2, 128], F16, tag="pt", name=f"ptz{li}_{t}")
                for fh in range(2):
                    nc.tensor.transpose(
                        out=ptz[:, fh, :],
                        in_=hs_sb[:, t, fh * 128 : (fh + 1) * 128],
                        identity=id16_sb[:],
                    )
                zT = sbp.tile([128, 2, 128], F16, tag="zT", name=f"zT{li}_{t}")
                nc.scalar.copy(out=zT[:], in_=ptz[:])
                do_tile_out(t, [(wlT_sb[li], mT), (wrT_sb[li], zT)], li + 1, li < 2)

            # ---------- conv layers ----------
            G = 1
            gctr = [0]
            for li in range(3):
                ztab = zfull[li]
                for g0 in range(0, ntile, G):
                    tiles = list(range(g0, min(g0 + G, ntile)))
                    done_t = {t: 0 for t in tiles}
                    total_t = {t: int(nchunks[t].sum()) for t in tiles}
                    pa = {
                        t: psA.tile([128, H], F32, tag="pa", name=f"pa{li}_{t}")[:]
                        for t in tiles
                    }
                    for b in range(NBANK):
                        run = []
                        for t in tiles:
                            cb = int(chunk_base[t, b])
                            for k in range(int(nchunks[t, b])):
                                run.append((cb + k, t))
                        for q0 in range(0, len(run), 8):
                            call = run[q0 : q0 + 8]
                            qn = len(call)
                            c0 = call[0][0]
                            g = gatp.tile([128, 8, H], F16, tag="g")
                            gctr[0] += 1
                            nc.gpsimd.dma_gather(
                                out_ap=g[:, :qn, :],
                                in_ap=ztab[b * BANK : (b + 1) * BANK, :],
                                idxs_ap=idx_sb[:, c0 * 8 : (c0 + qn) * 8],
                                num_idxs=qn * 128,
                                num_idxs_reg=qn * 128,
                                elem_size=H,
                                queue_num=0 if os.environ.get('GSAGE_Q0') else gctr[0] % 4,
                            )
                            for k, (col, t) in enumerate(call):
                                oh = ohp.tile([128, 128], F16, tag="oh")
                                nc.vector.tensor_tensor(
                                    out=oh[:],
                                    in0=dstl_sb[:, col : col + 1].to_broadcast([128, 128]),
                                    in1=iota_sb[:],
                                    op=mybir.AluOpType.is_equal,
                                )
                                done_t[t] += 1
                                nc.tensor.matmul(
                                    out=pa[t], lhsT=oh[:], rhs=g[:, k, :],
                                    start=(done_t[t] == 1),
                                    stop=(done_t[t] == total_t[t]),
                                )
                    for t in tiles:
                        _conv_tile_tail(li, t, pa[t])
                if debug:
                    for t in range(ntile):
                        lo = t * 128
                        n_t = min(128, SHARD - lo)
                        nc.sync.dma_start(out=dbg[li + 1][lo : lo + n_t, :],
                                          in_=hs_sb[:n_t, t, :])
                if li < 2:
                    nc.gpsimd.collective_compute(
                        "AllGather", mybir.AluOpType.bypass,
                        replica_groups=[list(range(NCORES))],
                        ins=[hs_dram[:].opt()], outs=[zfull[li + 1][:].opt()],
                    )

            # ---------- post layers + log_softmax (phase-batched) ----------
            lg_sb = hsp.tile([128, ntile, C], F32, name="lg_sb")
            esum_all = constp.tile([128, ntile], F32, name="esum_all")
            # pass 1: post1 + bn4 + relu
            for t in range(ntile):
                ptz = psT.tile([128, 2, 128], F16, tag="pt", name=f"pp1_{t}")
                for fh in range(2):
                    nc.tensor.transpose(
                        out=ptz[:, fh, :],
                        in_=hs_sb[:, t, fh * 128 : (fh + 1) * 128],
                        identity=id16_sb[:],
                    )
                zT = sbp.tile([128, 2, 128], F16, tag="zT", name=f"pz1_{t}")
                nc.scalar.copy(out=zT[:], in_=ptz[:])
                do_tile_out(t, [(p1T_sb, zT)], 4, False)
            # pass 2: logits + bias + max-shift into lg_sb
            for t in range(ntile):
                ptz2 = psT.tile([128, 2, 128], F16, tag="pt", name=f"pp2_{t}")
                for fh in range(2):
                    nc.tensor.transpose(
                        out=ptz2[:, fh, :],
                        in_=hs_sb[:, t, fh * 128 : (fh + 1) * 128],
                        identity=id16_sb[:],
                    )
                hT2 = sbp.tile([128, 2, 128], F16, tag="zT", name=f"ph2_{t}")
                nc.vector.tensor_copy(out=hT2[:], in_=ptz2[:])
                pl = psB.tile([128, 128], F32, tag="po", name=f"pl_{t}")
                for fi in range(2):
                    nc.tensor.matmul(
                        out=pl[:C, :], lhsT=p2T_sb[:, fi, :], rhs=hT2[:, fi, :],
                        start=(fi == 0), stop=(fi == 1),
                    )
                ltT = sbp.tile([C, 128], F32, tag="ltT", name=f"plt_{t}")
                nc.vector.tensor_scalar(
                    out=ltT[:], in0=pl[:C, :], scalar1=p2b_sb[:], scalar2=None,
                    op0=mybir.AluOpType.add,
                )
                pln = psB.tile([128, 128], F32, tag="po", name=f"pln_{t}")
                nc.tensor.matmul(out=pln[:, :C], lhsT=ltT[:], rhs=id32_sb[:C, :C],
                                 start=True, stop=True)
                nmx = vecp.tile([128, 1], F32, tag="nmx", name=f"pn_{t}")
                nc.vector.tensor_reduce(out=nmx[:], in_=pln[:, :C],
                                        axis=mybir.AxisListType.X,
                                        op=mybir.AluOpType.max, negate=True)
                nc.vector.tensor_scalar(
                    out=lg_sb[:, t, :], in0=pln[:, :C], scalar1=nmx[:], scalar2=None,
                    op0=mybir.AluOpType.add,
                )
            # pass 3: exp-sums (single ACT function)
            for t in range(ntile):
                etmp = vecp.tile([128, C], F32, tag="etmp", name=f"pe_{t}")
                nc.scalar.activation(out=etmp[:], in_=lg_sb[:, t, :], func=Exp,
                                     accum_out=esum_all[:, t : t + 1])
            # pass 4: one Ln for all tiles
            lse_all = constp.tile([128, ntile], F32, name="lse_all")
            nc.scalar.activation(out=lse_all[:], in_=esum_all[:], func=Ln)
            # pass 5: final subtract + store
            for t in range(ntile):
                fin = sbp.tile([128, C], F32, tag="fin", name=f"pf_{t}")
                nc.vector.tensor_scalar(
                    out=fin[:], in0=lg_sb[:, t, :],
                    scalar1=lse_all[:, t : t + 1], scalar2=None,
                    op0=mybir.AluOpType.subtract,
                )
                lo = t * 128
                n_t = min(128, SHARD - lo)
                if n_t > 0:
                    nc.sync.dma_start(out=out[lo : lo + n_t, :], in_=fin[:n_t, :])

    nc.compile()
    return nc


def kernel(**inputs):
    global LAST_EXEC_NS
    nchunks, chunk_base, total_chunks, in_maps = _preprocess(**inputs)
    nc = _build(nchunks, chunk_base, total_chunks)
    trace = bool(os.environ.get("GSAGE_TRACE"))
    res = run_bass_kernel_spmd(nc, in_maps, core_ids=list(range(NCORES)), trace=trace)
    LAST_EXEC_NS = res.exec_time_ns
    global LAST_RES
    LAST_RES = res
    return np.concatenate([res.results[c]["out"] for c in range(NCORES)], axis=0)


# revision 27
# speedup vs baseline: 1.0712x; 1.0712x over previous
"""GraphSAGE (3x SAGEConv mean-aggr + BN + MLP head) on 8 Trainium2 NeuronCores.

Strategy: node sharding (12500 nodes/core). Edges partitioned by dst core and
grouped by (dst-tile of 128, src-bank of 25000). Activation table is
AllGathered (fp16) each layer; per-edge source features are fetched with
gpsimd.dma_gather (4 SWDGE queues), segment-summed into PSUM via one-hot
matmuls, mean-scaled, then W_l/W_r matmuls with a folded BN+ReLU epilogue.
fp16 data path, f32 accumulation.
"""

import os
import numpy as np

from concourse import bass, bacc, mybir, tile
from concourse.bass_utils import run_bass_kernel_spmd
from concourse.library_config import mlp as MLP_LIB

# problem constants (hardcoded per spec)
N = 100000
E = 1600000
F_IN = 128
H = 256
C = 40
BN_EPS = 1e-5
NCORES = 8
SHARD = N // NCORES          # 12500
NTILE = SHARD // 128         # 98 dst tiles (the last one is 84 wide: 12500=97*128+84)
BANK = 25000                 # int16-addressable src bank size
NBANK = (N + BANK - 1) // BANK

F16 = mybir.dt.float16
F32 = mybir.dt.float32
I16 = mybir.dt.int16

LAST_EXEC_NS = None
LAST_RES = None


def _preprocess(x, edge_index, pre_w, pre_b, bn_params, lin_l_w, lin_l_b, lin_r_w,
                post1_w, post1_b, post2_w, post2_b):
    """Host-side sharding: per-core edge gather/one-hot metadata + weights."""
    ntile = (SHARD + 127) // 128
    src = np.asarray(edge_index[0], dtype=np.int64)
    dst = np.asarray(edge_index[1], dtype=np.int64)

    deg = np.bincount(dst, minlength=N).astype(np.float64)
    invdeg_full = (1.0 / np.maximum(deg, 1.0)).astype(np.float32)

    core_of = dst // SHARD
    counts = np.zeros((NCORES, ntile, NBANK), dtype=np.int64)
    per_core = []
    for c in range(NCORES):
        m = core_of == c
        s_c = src[m]
        d_c = dst[m] - c * SHARD
        t_c = d_c // 128
        b_c = s_c // BANK
        order = np.lexsort((b_c, t_c))
        s_c, d_c, t_c, b_c = s_c[order], d_c[order], t_c[order], b_c[order]
        key = t_c * NBANK + b_c
        cnt = np.bincount(key, minlength=ntile * NBANK).reshape(ntile, NBANK)
        counts[c] = cnt
        per_core.append((s_c, d_c, key))

    nchunks = (np.ceil(counts.max(axis=0) / 128.0)).astype(np.int64)  # [ntile, NBANK]
    total_chunks = int(nchunks.sum())

    idx_arr = np.zeros((NCORES, 128, total_chunks * 8), dtype=np.int16)
    dstl_arr = np.full((NCORES, 128, total_chunks), 255.0, dtype=np.float16)
    G = 1  # tiles per supertile group (shared gather calls per bank)
    chunk_base = np.zeros((ntile, NBANK), dtype=np.int64)
    acc = 0
    for g0 in range(0, ntile, G):
        for b in range(NBANK):
            for t in range(g0, min(g0 + G, ntile)):
                chunk_base[t, b] = acc
                acc += nchunks[t, b]

    for c in range(NCORES):
        s_c, d_c, _key = per_core[c]
        grp_cnt = counts[c].reshape(-1)
        grp_start = np.zeros(ntile * NBANK + 1, dtype=np.int64)
        np.cumsum(grp_cnt, out=grp_start[1:])
        for t in range(ntile):
            for b in range(NBANK):
                k = int(counts[c, t, b])
                if k == 0:
                    continue
                g0 = grp_start[t * NBANK + b]
                svals = (s_c[g0 : g0 + k] - b * BANK).astype(np.int16)
                dvals = (d_c[g0 : g0 + k] - t * 128).astype(np.float16)
                cb = int(chunk_base[t, b])
                i = np.arange(k)
                for g in range(8):
                    idx_arr[c, 16 * g + (i % 16), cb * 8 + (i // 16)] = svals
                dstl_arr[c, i % 128, cb + i // 128] = dvals
    # pads: idx stays 0 (valid row 0 of the bank), dstl stays 255 (no one-hot col)

    invd_arr = np.zeros((NCORES, 128, ntile), dtype=np.float32)
    for c in range(NCORES):
        v = np.zeros(ntile * 128, dtype=np.float32)
        v[:SHARD] = invdeg_full[c * SHARD : (c + 1) * SHARD]
        invd_arr[c, :, :] = v.reshape(ntile, 128).T

    # ---- weights (replicated) ----
    f = np.float32
    pre_w = np.asarray(pre_w, f); pre_b = np.asarray(pre_b, f)
    bn = np.asarray(bn_params, f)
    lin_l_w = np.asarray(lin_l_w, f); lin_l_b = np.asarray(lin_l_b, f)
    lin_r_w = np.asarray(lin_r_w, f)
    post1_w = np.asarray(post1_w, f); post1_b = np.asarray(post1_b, f)
    post2_w = np.asarray(post2_w, f); post2_b = np.asarray(post2_b, f)

    def lhsT2(w):  # w [fo, fi] -> [128, nfi, fo] fp16 (K-chunked transposed)
        fo, fi = w.shape
        nfi = fi // 128
        out = np.zeros((128, nfi, fo), dtype=np.float16)
        for kc in range(nfi):
            out[:, kc, :] = w[:, kc * 128 : (kc + 1) * 128].T.astype(np.float16)
        return out

    # BN folds: y -> relu(scale*y + shift)
    scales = np.zeros((5, H), f); shifts = np.zeros((5, H), f)
    biases = [pre_b, lin_l_b[0], lin_l_b[1], lin_l_b[2], post1_b]
    for k in range(5):
        g, be, m, v = bn[k, 0], bn[k, 1], bn[k, 2], bn[k, 3]
        s = g / np.sqrt(v + BN_EPS)
        scales[k] = s
        shifts[k] = s * (biases[k] - m) + be

    iota_row = np.tile(np.arange(128, dtype=np.float16)[None, :], (128, 1))
    ident16 = np.eye(128, dtype=np.float16)
    ident32 = np.eye(128, dtype=f)

    x_f16 = np.asarray(x, f).astype(np.float16)
    # pad shard to ntile*128 rows
    xpad = np.zeros((NCORES, ntile * 128, F_IN), dtype=np.float16)
    for c in range(NCORES):
        xpad[c, :SHARD] = x_f16[c * SHARD : (c + 1) * SHARD]

    common = {
        "iota_row": iota_row,
        "ident16": ident16,
        "ident32": ident32,
        "pre_wT": lhsT2(pre_w),
        "wlT": np.stack([lhsT2(lin_l_w[i]) for i in range(3)]),
        "wrT": np.stack([lhsT2(lin_r_w[i]) for i in range(3)]),
        "p1T": lhsT2(post1_w),
        "p2T": lhsT2(post2_w),
        "bn_scale": np.ascontiguousarray(scales.reshape(5, 2, 128).transpose(2, 0, 1)),
        "bn_shift": np.ascontiguousarray(shifts.reshape(5, 2, 128).transpose(2, 0, 1)),
        "p2b": np.ascontiguousarray(post2_b.reshape(C, 1)),
    }
    in_maps = []
    for c in range(NCORES):
        m = dict(common)
        m["x"] = np.ascontiguousarray(xpad[c])
        m["gidx"] = np.ascontiguousarray(idx_arr[c])
        m["dstl"] = np.ascontiguousarray(dstl_arr[c])
        m["invd"] = np.ascontiguousarray(invd_arr[c])
        in_maps.append(m)
    return nchunks, chunk_base, total_chunks, in_maps


def _build(nchunks, chunk_base, total_chunks):
    ntile = nchunks.shape[0]
    nc = bacc.Bacc("TRN2", target_bir_lowering=False, debug=False,
                   num_devices=NCORES, num_swdge_queues=4)
    dram_in = lambda n, s, d: nc.dram_tensor(n, s, d, kind="ExternalInput").ap()

    x_in = dram_in("x", [ntile * 128, F_IN], F16)
    gidx = dram_in("gidx", [128, total_chunks * 8], I16)
    dstl = dram_in("dstl", [128, total_chunks], F16)
    invd = dram_in("invd", [128, ntile], F32)
    iota_row = dram_in("iota_row", [128, 128], F16)
    ident16 = dram_in("ident16", [128, 128], F16)
    ident32 = dram_in("ident32", [128, 128], F32)
    pre_wT = dram_in("pre_wT", [128, 1, H], F16)
    wlT = dram_in("wlT", [3, 128, 2, H], F16)
    wrT = dram_in("wrT", [3, 128, 2, H], F16)
    p1T = dram_in("p1T", [128, 2, H], F16)
    p2T = dram_in("p2T", [128, 2, C], F16)
    bn_scale = dram_in("bn_scale", [128, 5, 2], F32)
    bn_shift = dram_in("bn_shift", [128, 5, 2], F32)
    p2b = dram_in("p2b", [C, 1], F32)
    out = nc.dram_tensor("out", [SHARD, C], F32, kind="ExternalOutput").ap()
    debug = bool(os.environ.get("GSAGE_DEBUG"))
    if debug:
        dbg = [nc.dram_tensor(f"dbg{i}", [SHARD, H], F16, kind="ExternalOutput").ap()
               for i in range(4)]
        dbgz = nc.dram_tensor("dbgz", [N, H], F16, kind="ExternalOutput").ap()

    Relu = mybir.ActivationFunctionType.Relu
    Copy = mybir.ActivationFunctionType.Copy
    Exp = mybir.ActivationFunctionType.Exp
    Ln = mybir.ActivationFunctionType.Ln
    Ident = mybir.ActivationFunctionType.Identity

    with tile.TileContext(nc) as tc:
        with (
            tc.tile_pool(name="const", bufs=1) as constp,
            tc.tile_pool(name="hs", bufs=1) as hsp,
            tc.tile_pool(name="gat", bufs=8) as gatp,
            tc.tile_pool(name="oh", bufs=8) as ohp,
            tc.tile_pool(name="sb", bufs=3) as sbp,
            tc.tile_pool(name="vec", bufs=4) as vecp,
            tc.tile_pool(name="psA", bufs=3, space="PSUM") as psA,
            tc.tile_pool(name="psB", bufs=2, space="PSUM") as psB,
            tc.tile_pool(name="psT", bufs=3, space="PSUM") as psT,
            tc.tile_pool(name="dram", bufs=1, space="DRAM") as dramp,
        ):
            nc.gpsimd.load_library(MLP_LIB)

            # ---- resident constants / inputs ----
            idx_sb = constp.tile([128, total_chunks * 8], I16)
            nc.sync.dma_start(out=idx_sb[:], in_=gidx[:])
            dstl_sb = constp.tile([128, total_chunks], F16)
            nc.sync.dma_start(out=dstl_sb[:], in_=dstl[:])
            invd_sb = constp.tile([128, ntile], F32)
            nc.sync.dma_start(out=invd_sb[:], in_=invd[:])
            iota_sb = constp.tile([128, 128], F16)
            nc.sync.dma_start(out=iota_sb[:], in_=iota_row[:])
            id16_sb = constp.tile([128, 128], F16)
            nc.sync.dma_start(out=id16_sb[:], in_=ident16[:])
            id32_sb = constp.tile([128, 128], F32)
            nc.sync.dma_start(out=id32_sb[:], in_=ident32[:])
            pre_wT_sb = constp.tile([128, 1, H], F16)
            nc.sync.dma_start(out=pre_wT_sb[:], in_=pre_wT[:])
            wlT_sb = [constp.tile([128, 2, H], F16, tag=f"wlT{i}", name=f"wlT{i}") for i in range(3)]
            wrT_sb = [constp.tile([128, 2, H], F16, tag=f"wrT{i}", name=f"wrT{i}") for i in range(3)]
            for i in range(3):
                nc.sync.dma_start(out=wlT_sb[i][:], in_=wlT[i])
                nc.sync.dma_start(out=wrT_sb[i][:], in_=wrT[i])
            p1T_sb = constp.tile([128, 2, H], F16)
            nc.sync.dma_start(out=p1T_sb[:], in_=p1T[:])
            p2T_sb = constp.tile([128, 2, C], F16)
            nc.sync.dma_start(out=p2T_sb[:], in_=p2T[:])
            bns_sb = constp.tile([128, 5, 2], F32)
            nc.sync.dma_start(out=bns_sb[:], in_=bn_scale[:])
            bnb_sb = constp.tile([128, 5, 2], F32)
            nc.sync.dma_start(out=bnb_sb[:], in_=bn_shift[:])
            p2b_sb = constp.tile([C, 1], F32)
            nc.sync.dma_start(out=p2b_sb[:], in_=p2b[:])

            # local shard activations, node-major fp16 [p, tile, feat]
            hs_sb = hsp.tile([128, ntile, H], F16)

            hs_dram = dramp.tile([SHARD, H], F16)
            zfull = [
                dramp.tile([N, H], F16, addr_space="Shared", tag=f"zfull{i}", name=f"zfull{i}")
                for i in range(3)
            ]


            def do_tile_out(t, terms, bnk, store_dram):
                """terms: list of (lhsT_sb [128,nfi,H], rhsT_sb [128,nfi,128]).
                Transposed matmul + fused BN+ReLU, transposed back into
                hs_sb[:, t, :]; optionally stages the row block to hs_dram."""
                po = psB.tile([128, 2, 128], F32, tag="po")
                total = 2 * sum(lh.shape[1] for lh, _ in terms)
                done = 0
                for fh in range(2):
                    for lh, rh in terms:
                        nfi = lh.shape[1]
                        for fi in range(nfi):
                            done += 1
                            nc.tensor.matmul(
                                out=po[:, fh, :],
                                lhsT=lh[:, fi, fh * 128 : (fh + 1) * 128],
                                rhs=rh[:, fi, :],
                                start=(done == 1 or (fh == 1 and done == total // 2 + 1)),
                                stop=(done == total // 2 or done == total),
                            )
                hT = sbp.tile([128, 2, 128], F16, tag="hT")
                for fh in range(2):
                    nc.scalar.activation(
                        out=hT[:, fh, :], in_=po[:, fh, :], func=Relu,
                        bias=bnb_sb[:, bnk, fh : fh + 1],
                        scale=bns_sb[:, bnk, fh : fh + 1],
                    )
                pt = psT.tile([128, 2, 128], F16, tag="pt")
                for fh in range(2):
                    nc.tensor.transpose(out=pt[:, fh, :], in_=hT[:, fh, :], identity=id16_sb[:])
                for fh in range(2):
                    nc.vector.tensor_copy(
                        out=hs_sb[:, t, fh * 128 : (fh + 1) * 128], in_=pt[:, fh, :]
                    )
                if store_dram:
                    lo = t * 128
                    n_t = min(128, SHARD - lo)
                    nc.sync.dma_start(
                        out=hs_dram[lo : lo + n_t, :], in_=hs_sb[:n_t, t, :]
                    )

            # ---------- pre layer: h = relu(bn(x @ pre_w.T + pre_b)) ----------
            for t in range(ntile):
                xt = sbp.tile([128, F_IN], F16, tag="xt")
                nc.sync.dma_start(out=xt[:], in_=x_in[t * 128 : (t + 1) * 128, :])
                ptx = psT.tile([128, 2, 128], F16, tag="pt")
                nc.tensor.transpose(out=ptx[:, 0, :], in_=xt[:], identity=id16_sb[:])
                xT = sbp.tile([128, 1, 128], F16, tag="xT")
                nc.vector.tensor_copy(out=xT[:, 0, :], in_=ptx[:, 0, :])
                do_tile_out(t, [(pre_wT_sb, xT)], 0, True)

            if debug:
                nc.sync.dma_start(out=dbg[0][:], in_=hs_dram[:])
            nc.gpsimd.collective_compute(
                "AllGather", mybir.AluOpType.bypass,
                replica_groups=[list(range(NCORES))],
                ins=[hs_dram[:].opt()], outs=[zfull[0][:].opt()],
            )


            if debug:
                for blk in range(0, N, 12500):
                    nc.sync.dma_start(out=dbgz[blk : blk + 12500, :],
                                      in_=zfull[0][blk : blk + 12500, :])
            def _conv_tile_tail(li, t, pa_t):
                mean_sb = sbp.tile([128, H], F16, tag="mean", name=f"mean{li}_{t}")
                nc.scalar.activation(
                    out=mean_sb[:], in_=pa_t, func=Copy,
                    scale=invd_sb[:, t : t + 1],
                )
                ptm = psT.tile([128, 2, 128], F16, tag="pt", name=f"ptm{li}_{t}")
                for fh in range(2):
                    nc.tensor.transpose(
                        out=ptm[:, fh, :],
                        in_=mean_sb[:, fh * 128 : (fh + 1) * 128],
                        identity=id16_sb[:],
                    )
                mT = sbp.tile([128, 2, 128], F16, tag="mT", name=f"mT{li}_{t}")
                nc.vector.tensor_copy(out=mT[:], in_=ptm[:])
                ptz = psT.tile([128, 2, 128], F16, tag="pt", name=f"ptz{li}_{t}")
                for fh in range(2):
                    nc.tensor.transpose(
                        out=ptz[:, fh, :],
                        in_=hs_sb[:, t, fh * 128 : (fh + 1) * 128],
                        identity=id16_sb[:],
                    )
                zT = sbp.tile([128, 2, 128], F16, tag="zT", name=f"zT{li}_{t}")
                nc.scalar.copy(out=zT[:], in_=ptz[:])
                do_tile_out(t, [(wlT_sb[li], mT), (wrT_sb[li], zT)], li + 1, li < 2)

            # ---------- conv layers ----------
            G = 1
            gctr = [0]
            for li in range(3):
                ztab = zfull[li]
                for g0 in range(0, ntile, G):
                    tiles = list(range(g0, min(g0 + G, ntile)))
                    done_t = {t: 0 for t in tiles}
                    total_t = {t: int(nchunks[t].sum()) for t in tiles}
                    pa = {
                        t: psA.tile([128, H], F32, tag="pa", name=f"pa{li}_{t}")[:]
                        for t in tiles
                    }
                    for b in range(NBANK):
                        run = []
                        for t in tiles:
                            cb = int(chunk_base[t, b])
                            for k in range(int(nchunks[t, b])):
                                run.append((cb + k, t))
                        for q0 in range(0, len(run), 8):
                            call = run[q0 : q0 + 8]
                            qn = len(call)
                            c0 = call[0][0]
                            g = gatp.tile([128, 8, H], F16, tag="g")
                            gctr[0] += 1
                            nc.gpsimd.dma_gather(
                                out_ap=g[:, :qn, :],
                                in_ap=ztab[b * BANK : (b + 1) * BANK, :],
                                idxs_ap=idx_sb[:, c0 * 8 : (c0 + qn) * 8],
                                num_idxs=qn * 128,
                                num_idxs_reg=qn * 128,
                                elem_size=H,
                                queue_num=0 if os.environ.get('GSAGE_Q0') else gctr[0] % 4,
                            )
                            for k, (col, t) in enumerate(call):
                                oh = ohp.tile([128, 128], F16, tag="oh")
                                nc.vector.tensor_tensor(
                                    out=oh[:],
                                    in0=dstl_sb[:, col : col + 1].to_broadcast([128, 128]),
                                    in1=iota_sb[:],
                                    op=mybir.AluOpType.is_equal,
                                )
                                done_t[t] += 1
                                nc.tensor.matmul(
                                    out=pa[t], lhsT=oh[:], rhs=g[:, k, :],
                                    start=(done_t[t] == 1),
                                    stop=(done_t[t] == total_t[t]),
                                )
                    for t in tiles:
                        _conv_tile_tail(li, t, pa[t])
                if debug:
                    for t in range(ntile):
                        lo = t * 128
                        n_t = min(128, SHARD - lo)
                        nc.sync.dma_start(out=dbg[li + 1][lo : lo + n_t, :],
                                          in_=hs_sb[:n_t, t, :])
                if li < 2:
                    nc.gpsimd.collective_compute(
                        "AllGather", mybir.AluOpType.bypass,
                        replica_groups=[list(range(NCORES))],
                        ins=[hs_dram[:].opt()], outs=[zfull[li + 1][:].opt()],
                    )

            # ---------- post layers + log_softmax (phase-batched) ----------
            lg_sb = hsp.tile([128, ntile, C], F32, name="lg_sb")
            esum_all = constp.tile([128, ntile], F32, name="esum_all")
            # pass 1: post1 + bn4 + relu
            for t in range(ntile):
                ptz = psT.tile([128, 2, 128], F16, tag="pt", name=f"pp1_{t}")
                for fh in range(2):
                    nc.tensor.transpose(
                        out=ptz[:, fh, :],
                        in_=hs_sb[:, t, fh * 128 : (fh + 1) * 128],
                        identity=id16_sb[:],
                    )
                zT = sbp.tile([128, 2, 128], F16, tag="zT", name=f"pz1_{t}")
                nc.scalar.copy(out=zT[:], in_=ptz[:])
                do_tile_out(t, [(p1T_sb, zT)], 4, False)
            # pass 2: logits + bias + max-shift into lg_sb
            for t in range(ntile):
                ptz2 = psT.tile([128, 2, 128], F16, tag="pt", name=f"pp2_{t}")
                for fh in range(2):
                    nc.tensor.transpose(
                        out=ptz2[:, fh, :],
                        in_=hs_sb[:, t, fh * 128 : (fh + 1) * 128],
                        identity=id16_sb[:],
                    )
                hT2 = sbp.tile([128, 2, 128], F16, tag="zT", name=f"ph2_{t}")
                nc.vector.tensor_copy(out=hT2[:], in_=ptz2[:])
                pl = psB.tile([128, 128], F32, tag="po", name=f"pl_{t}")
                for fi in range(2):
                    nc.tensor.matmul(
                        out=pl[:C, :], lhsT=p2T_sb[:, fi, :], rhs=hT2[:, fi, :],
                        start=(fi == 0), stop=(fi == 1),
                    )
                ltT = sbp.tile([C, 128], F32, tag="ltT", name=f"plt_{t}")
                nc.vector.tensor_scalar(
                    out=ltT[:], in0=pl[:C, :], scalar1=p2b_sb[:], scalar2=None,
                    op0=mybir.AluOpType.add,
                )
                pln = psB.tile([128, 128], F32, tag="po", name=f"pln_{t}")
                nc.tensor.matmul(out=pln[:, :C], lhsT=ltT[:], rhs=id32_sb[:C, :C],
                                 start=True, stop=True)
                nmx = vecp.tile([128, 1], F32, tag="nmx", name=f"pn_{t}")
                nc.vector.tensor_reduce(out=nmx[:], in_=pln[:, :C],
                                        axis=mybir.AxisListType.X,
                                        op=mybir.AluOpType.max, negate=True)
                nc.vector.tensor_scalar(
                    out=lg_sb[:, t, :], in0=pln[:, :C], scalar1=nmx[:], scalar2=None,
                    op0=mybir.AluOpType.add,
                )
            # pass 3: exp-sums (single ACT function)
            for t in range(ntile):
                etmp = vecp.tile([128, C], F32, tag="etmp", name=f"pe_{t}")
                nc.scalar.activation(out=etmp[:], in_=lg_sb[:, t, :], func=Exp,
                                     accum_out=esum_all[:, t : t + 1])
            # pass 4: one Ln for all tiles
            lse_all = constp.tile([128, ntile], F32, name="lse_all")
            nc.scalar.activation(out=lse_all[:], in_=esum_all[:], func=Ln)
            # pass 5: final subtract + store
            for t in range(ntile):
                fin = sbp.tile([128, C], F32, tag="fin", name=f"pf_{t}")
                nc.vector.tensor_scalar(
                    out=fin[:], in0=lg_sb[:, t, :],
                    scalar1=lse_all[:, t : t + 1], scalar2=None,
                    op0=mybir.AluOpType.subtract,
                )
                lo = t * 128
                n_t = min(128, SHARD - lo)
                if n_t > 0:
                    nc.sync.dma_start(out=out[lo : lo + n_t, :], in_=fin[:n_t, :])

    nc.compile()
    return nc


def kernel(**inputs):
    global LAST_EXEC_NS
    nchunks, chunk_base, total_chunks, in_maps = _preprocess(**inputs)
    nc = _build(nchunks, chunk_base, total_chunks)
    trace = bool(os.environ.get("GSAGE_TRACE"))
    res = run_bass_kernel_spmd(nc, in_maps, core_ids=list(range(NCORES)), trace=trace)
    LAST_EXEC_NS = res.exec_time_ns
    global LAST_RES
    LAST_RES = res
    return np.concatenate([res.results[c]["out"] for c in range(NCORES)], axis=0)
